# revision 20
# baseline (speedup 1.0000x reference)
# GraphSAGE (3-layer, mean aggregation) on 8 Trainium2 NeuronCores.
#
# Sharding: nodes are split into 8 contiguous ranges (6250 per core); edges
# are partitioned by destination node so each core's scatter-adds stay local.
# Each layer's feature table is replicated per core (layer 0: shipped as two
# pre-permuted half-tables; layers 1-2: two AllGathers per boundary), so the
# per-edge source gathers are local HBM reads (int16 indices per half).
#
# Per core, per layer, for each chunk of 128 destination nodes:
#   - dma_gather the (dst-sorted) edges' source rows from the bf16 table
#   - build a selection matrix S[e, j] = (dst_local[e] == j) * 1/deg per
#     128-edge tile with one DVE tensor_scalar (bf16, 2x mode) against an
#     iota row, and accumulate aggrT += msg_tile^T-contract S on the PE into
#     PSUM ([feat, node] transposed mean aggregation)
#   - transpose the chunk's own rows with the PE, then
#     h = relu(aggrT^T @ w_l + own @ w_r), ScalarE draining PSUM into the
#     next layer's SBUF-resident own-feature table
# Collective overlap: the table halves are chunk-aligned; the half whose h
# rows finish first AllGathers mid-layer (hidden under the remaining
# chunks), and the late half's AllGather hides under the next layer's
# pass-1, which processes only tiles sourced from the early half (partial
# aggregations parked in SBUF and re-injected into PSUM via an identity
# matmul in pass 2). Own-row transposes and the w_r matmul also run in
# pass 1 (they need no table). Layer 0 runs single-pass (tables are inputs).
# Host-side preprocessing (edge sort/partition, padding to an SPMD-uniform
# tile layout, index tables) is numpy; inputs shipped minimal (bf16 aux,
# idx tables wrapped [16, cols] and replicated on device, bf16 output).
import os
import sys

import numpy as np

for _p in ("/opt/trn_rl_repo", "/root/.axon_site/_ro/trn_rl_repo"):
    if _p not in sys.path and os.path.isdir(_p):
        sys.path.append(_p)

import ml_dtypes  # noqa: E402

from concourse import bacc, bass, mybir, tile  # noqa: E402
from concourse.bass_utils import axon_active, run_bass_kernel_spmd  # noqa: E402
from concourse.masks import make_identity  # noqa: E402

P = 128
BF16 = mybir.dt.bfloat16
F32 = mybir.dt.float32
I16 = mybir.dt.int16
NP_BF16 = ml_dtypes.bfloat16


class Cfg:
    def __init__(self, n_nodes=50000, n_cores=8, d_in=128, d_hid=128, d_out=64,
                 split_chunks=20):
        assert n_nodes % n_cores == 0
        self.N = n_nodes
        self.NC = n_cores
        self.NPC = n_nodes // n_cores
        self.D_IN = d_in
        self.D_HID = d_hid
        self.D_OUT = d_out
        self.NCH = (self.NPC + P - 1) // P
        self.SC = split_chunks              # chunks in half A
        self.RA = split_chunks * P          # local rows in half A
        self.RB = self.NPC - self.RA        # local rows in half B
        self.TA = self.RA * n_cores         # table-A rows
        self.TB = self.RB * n_cores         # table-B rows
        assert self.TA < 2**15 and self.TB < 2**15


def preprocess(cfg: Cfg, src: np.ndarray, dst: np.ndarray):
    """Sort edges by dst, group per (chunk, table-half), pad to SPMD-uniform
    tiles, emit SBUF-layout index/dst/invd arrays ([16, cols] idx wrap)."""
    N, NPC, NCH, NC = cfg.N, cfg.NPC, cfg.NCH, cfg.NC
    RA, RB = cfg.RA, cfg.RB

    deg = np.bincount(dst, minlength=N)
    invd_node = (1.0 / np.maximum(deg, 1)).astype(np.float32)

    order = np.argsort(dst, kind="stable")
    s_src = src[order]
    s_dst = dst[order]

    lo_cnt = np.zeros((NC, NCH), dtype=np.int64)
    hi_cnt = np.zeros((NC, NCH), dtype=np.int64)
    bounds = np.empty((NC, NCH + 1), dtype=np.int64)
    for i in range(NC):
        for c in range(NCH + 1):
            bounds[i, c] = np.searchsorted(s_dst, i * NPC + min(c * P, NPC), "left")
        for c in range(NCH):
            e0, e1 = bounds[i, c], bounds[i, c + 1]
            in_a = (s_src[e0:e1] % NPC) < RA
            lo_cnt[i, c] = int(np.count_nonzero(in_a))
            hi_cnt[i, c] = (e1 - e0) - lo_cnt[i, c]

    cdiv = lambda a, b: -(-a // b)
    T_a = [int(cdiv(int(lo_cnt[:, c].max()), P)) for c in range(NCH)]
    T_b = [int(cdiv(int(hi_cnt[:, c].max()), P)) for c in range(NCH)]
    TT = sum(T_a) + sum(T_b)

    # slot layout: [all chunks' A-tiles in chunk order] then [all B-tiles]
    # (pass 1 = A tiles, pass 2 = B tiles)
    a_off = np.cumsum([0] + T_a)  # tile offsets within the A region
    b_off = np.cumsum([0] + T_b)
    NTA, NTB = a_off[-1], b_off[-1]
    SLOTS = TT * P

    idx16 = np.zeros((NC, SLOTS), dtype=np.int16)
    dstloc = np.full((NC, SLOTS), -1.0, dtype=np.float32)
    invd = np.zeros((NC, SLOTS), dtype=np.float32)

    for i in range(NC):
        for c in range(NCH):
            e0, e1 = bounds[i, c], bounds[i, c + 1]
            seg_src = s_src[e0:e1]
            seg_dst = s_dst[e0:e1]
            s_i, s_r = seg_src // NPC, seg_src % NPC
            in_a = s_r < RA
            tidx = np.where(in_a, s_i * RA + s_r, s_i * RB + (s_r - RA))
            base = i * NPC + c * P
            for sel, off_tiles in ((True, a_off[c]), (False, NTA + b_off[c])):
                m = in_a == sel
                n = int(m.sum())
                pos = off_tiles * P
                idx16[i, pos : pos + n] = tidx[m].astype(np.int16)
                dstloc[i, pos : pos + n] = (seg_dst[m] - base).astype(np.float32)
                invd[i, pos : pos + n] = invd_node[seg_dst[m]]

    # idx wrap: slot j -> [j % 16, j // 16]; shipped once (16 rows), the
    # kernel replicates it to all eight 16-partition groups.
    idx16_sb = np.ascontiguousarray(
        idx16.reshape(NC, SLOTS // 16, 16).transpose(0, 2, 1)
    )  # [NC, 16, SLOTS//16]
    dst_sb = np.ascontiguousarray(
        dstloc.reshape(NC, TT, P).transpose(0, 2, 1)
    ).astype(NP_BF16)
    inv_sb = np.ascontiguousarray(
        invd.reshape(NC, TT, P).transpose(0, 2, 1)
    ).astype(NP_BF16)

    return T_a, T_b, idx16_sb, dst_sb, inv_sb


def build(cfg: Cfg, T_a, T_b, n_gather_queues=1, maxi=512, repeat=1,
          skip_collectives=False, s_pool_mod=0):
    N, NPC, NCH, NC = cfg.N, cfg.NPC, cfg.NCH, cfg.NC
    D_IN, D_HID, D_OUT = cfg.D_IN, cfg.D_HID, cfg.D_OUT
    SC, RA, RB, TA, TB = cfg.SC, cfg.RA, cfg.RB, cfg.TA, cfg.TB
    TT = sum(T_a) + sum(T_b)
    NTA = sum(T_a)
    TAMAX = max(T_a)
    TBMAX = max(T_b)
    a_off = np.cumsum([0] + T_a)
    b_off = np.cumsum([0] + T_b)

    nc = bacc.Bacc(
        "TRN2",
        target_bir_lowering=False,
        debug=not axon_active(),
        num_devices=NC,
        num_swdge_queues=n_gather_queues,
    )

    xown_d = nc.dram_tensor("xown", [NPC, D_IN], BF16, kind="ExternalInput")
    xa_d = nc.dram_tensor("xa", [TA, D_IN], BF16, kind="ExternalInput")
    xb_d = nc.dram_tensor("xb", [TB, D_IN], BF16, kind="ExternalInput")
    idx_d = nc.dram_tensor("idx16", [16, TT * 8], I16, kind="ExternalInput")
    dst_d = nc.dram_tensor("dstloc", [P, TT], BF16, kind="ExternalInput")
    inv_d = nc.dram_tensor("invd", [P, TT], BF16, kind="ExternalInput")
    w_d = {}
    for li, (din, dout) in enumerate(((D_IN, D_HID), (D_HID, D_HID), (D_HID, D_OUT))):
        w_d[f"wl{li}"] = nc.dram_tensor(f"wl{li}", [din, dout], BF16, kind="ExternalInput")
        w_d[f"wr{li}"] = nc.dram_tensor(f"wr{li}", [din, dout], BF16, kind="ExternalInput")
    out_d = nc.dram_tensor("out", [NPC, D_OUT], BF16, kind="ExternalOutput")

    from contextlib import ExitStack

    with tile.TileContext(nc) as tc, ExitStack() as stk:
        const = stk.enter_context(tc.tile_pool(name="const", bufs=1))
        iota_b = const.tile([P, P], BF16, name="iota_b")
        iota_i = const.tile([P, P], mybir.dt.int32, name="iota_i")
        nc.gpsimd.iota(iota_i[:], pattern=[[1, P]], base=0, channel_multiplier=0)
        nc.vector.tensor_copy(iota_b[:], iota_i[:])
        ident = const.tile([P, P], BF16, name="ident")
        make_identity(nc, ident[:])

        idx_t = const.tile([P, TT * 8], I16, name="idx_t")
        for g in range(8):
            nc.sync.dma_start(idx_t[16 * g : 16 * (g + 1), :], idx_d[:])
        dst_b = const.tile([P, TT], BF16, name="dst_b")
        nc.sync.dma_start(dst_b[:], dst_d[:])
        inv_b = const.tile([P, TT], BF16, name="inv_b")
        nc.sync.dma_start(inv_b[:], inv_d[:])
        dst_t = const.tile([P, TT], F32, name="dst_t")
        nc.vector.tensor_copy(dst_t[:], dst_b[:])
        inv_t = const.tile([P, TT], F32, name="inv_t")
        nc.vector.tensor_copy(inv_t[:], inv_b[:])

        w_t = {}
        for k, d in w_d.items():
            w_t[k] = const.tile(list(d.shape), d.dtype, name=f"{k}_t")
            nc.sync.dma_start(w_t[k][:], d[:])

        # own-feature tables (SBUF-resident), one per layer boundary
        own_all = [
            const.tile([P, NCH, D_IN if li == 0 else D_HID], BF16,
                       name=f"own_all{li}")
            for li in range(3)
        ]
        # layer-0 own rows: row r -> [r % 128, r // 128, :]
        nc.sync.dma_start(
            own_all[0][:, : NPC // P, :],
            xown_d[0 : (NPC // P) * P, :].rearrange("(c p) f -> p c f", p=P),
        )
        if NPC % P:
            nc.sync.dma_start(
                own_all[0][: NPC % P, NPC // P, :],
                xown_d[(NPC // P) * P :, :],
            )
        out_all = const.tile([P, NCH, D_OUT], BF16, name="out_all")
        partials = [
            const.tile([P, NCH, P], BF16, name=f"partial{i}") for i in range(2)
        ]
        hparts = [
            const.tile([P, NCH, D_HID], BF16, name=f"hpart{i}") for i in range(2)
        ]

        dram = stk.enter_context(tc.tile_pool(name="dram", bufs=1, space="DRAM"))

        msgp = stk.enter_context(tc.tile_pool(name="msg", bufs=6))
        sgp = stk.enter_context(tc.tile_pool(name="sel", bufs=4))
        wk = stk.enter_context(tc.tile_pool(name="wk", bufs=4))
        ps_ag = stk.enter_context(tc.tile_pool(name="ps_ag", bufs=3, space="PSUM"))
        ps_tr = stk.enter_context(tc.tile_pool(name="ps_tr", bufs=2, space="PSUM"))
        ps_h = stk.enter_context(tc.tile_pool(name="ps_h", bufs=2, space="PSUM"))

        gq = [0]
        nreg = {}

        def gather(out_ap, tab_ap, col0, n_idx):
            qn = gq[0] % n_gather_queues
            gq[0] += 1
            for off in range(0, n_idx, maxi):
                n = min(maxi, n_idx - off)
                t0, t1 = off // P, (off + n) // P
                if n not in nreg:
                    nreg[n] = nc.gpsimd.to_reg(n)
                nc.gpsimd.dma_gather(
                    out_ap[:, t0:t1, :],
                    tab_ap,
                    idx_t[:, col0 + off // 16 : col0 + (off + n) // 16],
                    num_idxs=n,
                    num_idxs_reg=nreg[n],
                    elem_size=out_ap.shape[-1],
                    queue_num=qn,
                )

        scount = [0]

        def sbuild_into(s_ap, til):
            scount[0] += 1
            eng = (
                nc.gpsimd
                if s_pool_mod and scount[0] % s_pool_mod == 0
                else nc.vector
            )
            eng.tensor_scalar(
                s_ap,
                iota_b[:],
                dst_t[:, til : til + 1],
                inv_t[:, til : til + 1],
                mybir.AluOpType.is_equal,
                mybir.AluOpType.mult,
            )

        # S selection matrices depend only on (dstloc, invd), which are
        # layer-independent: build them once (first layer of the first rep),
        # spill to DRAM, and DMA-reload afterwards instead of re-running the
        # DVE builds.
        s_store = dram.tile([P, TT * P], BF16, name="s_store")
        SGMAX = max(TAMAX, TBMAX)

        def s_group(build, til0, T):
            sg = sgp.tile([P, SGMAX, P], BF16, tag="Sg")
            view = s_store[:, til0 * P : (til0 + T) * P].rearrange(
                "p (t j) -> p t j", j=P
            )
            if build:
                for t in range(T):
                    sbuild_into(sg[:, t, :], til0 + t)
                nc.sync.dma_start(view, sg[:, :T, :])
            else:
                nc.sync.dma_start(sg[:, :T, :], view)
            return sg

        for rep in range(repeat):
            # per-rep DRAM tables (AllGather outputs + inputs)
            h_own = [
                dram.tile([NPC, D_HID], BF16, name=f"h_own{li}_r{rep}")
                for li in range(2)
            ]
            tabs = [(xa_d, xb_d)]  # per layer: (tab_a, tab_b)
            for li in range(1, 3):
                tabs.append(
                    (
                        dram.tile([TA, D_HID], BF16, name=f"tabA{li}_r{rep}",
                                  addr_space="Shared"),
                        dram.tile([TB, D_HID], BF16, name=f"tabB{li}_r{rep}",
                                  addr_space="Shared"),
                    )
                )
            out_t = out_d if rep == repeat - 1 else dram.tile(
                [NPC, D_OUT], BF16, name=f"oscr_r{rep}"
            )

            def ag(ins_ap, out_tile):
                if skip_collectives:
                    return
                nc.gpsimd.collective_compute(
                    "AllGather",
                    mybir.AluOpType.bypass,
                    replica_groups=[list(range(NC))],
                    ins=[ins_ap],
                    outs=[out_tile.opt()],
                )

            # per-tile-group accessors: half 0 = A, half 1 = B
            def tiles_of(half, c):
                return T_a[c] if half == 0 else T_b[c]

            def til0_of(half, c):
                return a_off[c] if half == 0 else NTA + b_off[c]

            def msgshape(half):
                return TAMAX if half == 0 else TBMAX

            def emit_htab(layer, half, h_own, nxt, dout):
                """Batched h write for one table half + its AllGather."""
                if half == 0:
                    nc.sync.dma_start(
                        h_own[0:RA, :].rearrange("(c p) f -> p c f", p=P),
                        nxt[:, 0:SC, :dout],
                    )
                    ag(h_own[0:RA, :], tabs[layer + 1][0])
                else:
                    nc.sync.dma_start(
                        h_own[RA : RA + (NCH - SC - 1) * P, :]
                        .rearrange("(c p) f -> p c f", p=P),
                        nxt[:, SC : NCH - 1, :dout],
                    )
                    nc.sync.dma_start(
                        h_own[(NCH - 1) * P : NPC, :],
                        nxt[: NPC - (NCH - 1) * P, NCH - 1, :dout],
                    )
                    ag(h_own[RA:NPC, :], tabs[layer + 1][1])

            # parity plan: boundary AG that fires early alternates A, B, so
            # the late AG always hides under the next layer's pass 1.
            early = [0, 1]          # early half per boundary layer (0, 1)
            p1_half = [0, 0, 1]     # pass-1 tile half per layer
            for layer in range(3):
                din = D_IN if layer == 0 else D_HID
                dout = D_HID if layer < 2 else D_OUT
                wl_t = w_t[f"wl{layer}"]
                wr_t = w_t[f"wr{layer}"]
                own = own_all[layer]
                h1 = p1_half[layer]
                h2 = 1 - h1
                tab1 = tabs[layer][h1]
                tab2 = tabs[layer][h2]
                if layer < 2:
                    # chunk order: the early half's chunks first
                    p2_order = (
                        list(range(NCH)) if early[layer] == 0
                        else list(range(SC, NCH)) + list(range(SC))
                    )
                else:
                    p2_order = list(range(NCH))
                partial = partials[layer % 2]
                hpart = hparts[layer % 2]
                merged = layer == 0  # both tables ready at start: single pass

                # ---- pass 1: h1 tiles -> partial aggT; own -> hpart ----
                if not merged:
                    for c in range(NCH):
                        T1 = tiles_of(h1, c)
                        if T1:
                            msg_t = msgp.tile([P, msgshape(h1), din], BF16,
                                              tag="msg1")
                            gather(msg_t[:, :T1, :], tab1[:],
                                   til0_of(h1, c) * 8, T1 * P)
                            sg = s_group(False, til0_of(h1, c), T1)
                            agg_ps = ps_ag.tile([P, P], F32, tag="agg")
                            for t in range(T1):
                                nc.tensor.matmul(
                                    agg_ps[:],
                                    lhsT=msg_t[:, t, :],
                                    rhs=sg[:, t, :],
                                    start=(t == 0),
                                    stop=(t == T1 - 1),
                                )
                            nc.scalar.activation(
                                partial[:, c, :], agg_ps[:],
                                mybir.ActivationFunctionType.Copy,
                            )
                        # own-row transpose + lin_r matmul (table-free work)
                        xT_ps = ps_tr.tile([P, P], BF16, tag="xT")
                        nc.tensor.transpose(xT_ps[:], own[:, c, :din], ident[:])
                        xT = wk.tile([P, P], BF16, tag="xT_sb")
                        nc.scalar.activation(
                            xT[:], xT_ps[:], mybir.ActivationFunctionType.Copy
                        )
                        hp_ps = ps_h.tile([P, dout], F32, tag="h")
                        nc.tensor.matmul(hp_ps[:], lhsT=xT[:, :din], rhs=wr_t[:],
                                         start=True, stop=True)
                        nc.scalar.activation(
                            hpart[:, c, :dout], hp_ps[:],
                            mybir.ActivationFunctionType.Copy,
                        )

                # ---- pass 2: h2 tiles + weights + output ----
                done = [0, 0]  # chunks completed per half
                for c in p2_order:
                    T2 = tiles_of(h2, c)
                    agg_ps = ps_ag.tile([P, P], F32, tag="agg")
                    started = False
                    if merged:
                        T1 = tiles_of(h1, c)
                        if T1:
                            msg_t = msgp.tile([P, msgshape(h1), din], BF16,
                                              tag="msg1")
                            gather(msg_t[:, :T1, :], tab1[:],
                                   til0_of(h1, c) * 8, T1 * P)
                            sg = s_group(rep == 0, til0_of(h1, c), T1)
                            for t in range(T1):
                                nc.tensor.matmul(
                                    agg_ps[:],
                                    lhsT=msg_t[:, t, :],
                                    rhs=sg[:, t, :],
                                    start=(t == 0),
                                    stop=False,
                                )
                            started = True
                    elif tiles_of(h1, c) > 0:
                        nc.tensor.matmul(
                            agg_ps[:], lhsT=ident[:], rhs=partial[:, c, :],
                            start=True, stop=(T2 == 0),
                        )
                        started = True
                    if T2:
                        msg_t = msgp.tile([P, msgshape(h2), din], BF16, tag="msg2")
                        gather(msg_t[:, :T2, :], tab2[:], til0_of(h2, c) * 8,
                               T2 * P)
                        sg = s_group(merged and rep == 0, til0_of(h2, c), T2)
                        for t in range(T2):
                            nc.tensor.matmul(
                                agg_ps[:],
                                lhsT=msg_t[:, t, :],
                                rhs=sg[:, t, :],
                                start=not started and t == 0,
                                stop=(t == T2 - 1),
                            )
                    aggT = wk.tile([P, P], BF16, tag="aggT")
                    nc.scalar.activation(
                        aggT[:], agg_ps[:], mybir.ActivationFunctionType.Copy
                    )

                    h_ps = ps_h.tile([P, dout], F32, tag="h")
                    if merged:
                        xT_ps = ps_tr.tile([P, P], BF16, tag="xT")
                        nc.tensor.transpose(xT_ps[:], own[:, c, :din], ident[:])
                        xT = wk.tile([P, P], BF16, tag="xT_sb")
                        nc.scalar.activation(
                            xT[:], xT_ps[:], mybir.ActivationFunctionType.Copy
                        )
                        nc.tensor.matmul(h_ps[:], lhsT=aggT[:], rhs=wl_t[:],
                                         start=True, stop=False)
                        nc.tensor.matmul(h_ps[:], lhsT=xT[:, :din], rhs=wr_t[:],
                                         start=False, stop=True)
                    else:
                        nc.tensor.matmul(h_ps[:], lhsT=ident[:],
                                         rhs=hpart[:, c, :dout],
                                         start=True, stop=False)
                        nc.tensor.matmul(h_ps[:], lhsT=aggT[:], rhs=wl_t[:],
                                         start=False, stop=True)

                    if layer < 2:
                        nc.scalar.activation(
                            own_all[layer + 1][:, c, :dout], h_ps[:],
                            mybir.ActivationFunctionType.Relu,
                        )
                    else:
                        nc.scalar.activation(
                            out_all[:, c, :], h_ps[:],
                            mybir.ActivationFunctionType.Copy,
                        )

                    if layer < 2:
                        hf = 0 if c < SC else 1
                        done[hf] += 1
                        if done[hf] == (SC if hf == 0 else NCH - SC):
                            emit_htab(layer, hf, h_own[layer],
                                      own_all[layer + 1], dout)

                # final output write
                if layer == 2:
                    nc.sync.dma_start(
                        out_t[0 : (NCH - 1) * P, :].rearrange(
                            "(c p) f -> p c f", p=P),
                        out_all[:, 0 : NCH - 1, :],
                    )
                    nc.sync.dma_start(
                        out_t[(NCH - 1) * P : NPC, :],
                        out_all[: NPC - (NCH - 1) * P, NCH - 1, :],
                    )

    nc.compile()
    return nc


def run(cfg: Cfg, inputs: dict, trace=False, tmpdir=None, **bkw):
    x = np.asarray(inputs["x"], dtype=np.float32)
    ei = np.asarray(inputs["edge_index"])
    src = ei[0].astype(np.int64)
    dst = ei[1].astype(np.int64)

    T_a, T_b, idx16_sb, dst_sb, inv_sb = preprocess(cfg, src, dst)
    nc = build(cfg, T_a, T_b, **bkw)

    x_bf = x.astype(NP_BF16)
    x3 = x_bf.reshape(cfg.NC, cfg.NPC, cfg.D_IN)
    xa = np.ascontiguousarray(x3[:, : cfg.RA, :]).reshape(cfg.TA, cfg.D_IN)
    xb = np.ascontiguousarray(x3[:, cfg.RA :, :]).reshape(cfg.TB, cfg.D_IN)
    in_maps = []
    for i in range(cfg.NC):
        m = {
            "xown": np.ascontiguousarray(x_bf[i * cfg.NPC : (i + 1) * cfg.NPC]),
            "xa": xa,
            "xb": xb,
            "idx16": idx16_sb[i],
            "dstloc": dst_sb[i],
            "invd": inv_sb[i],
        }
        for li in range(3):
            m[f"wl{li}"] = np.asarray(inputs[f"w_l{li}"], np.float32).astype(NP_BF16)
            m[f"wr{li}"] = np.asarray(inputs[f"w_r{li}"], np.float32).astype(NP_BF16)
        in_maps.append(m)

    results = run_bass_kernel_spmd(
        nc, in_maps, core_ids=list(range(cfg.NC)), trace=trace, tmpdir=tmpdir
    )
    outs = [np.asarray(r["out"]).astype(np.float32) for r in results.results]
    return np.concatenate(outs, axis=0), results, in_maps, nc


def kernel(**inputs) -> np.ndarray:
    cfg = Cfg()
    out, *_ = run(cfg, inputs)
    return out


# revision 25
# speedup vs baseline: 1.0934x; 1.0934x over previous
# GraphSAGE (3-layer, mean aggregation) on 8 Trainium2 NeuronCores.
#
# Sharding: nodes are split into 8 contiguous ranges (6250 per core); edges
# are partitioned by destination node so each core's scatter-adds stay local.
# Each layer's feature table is replicated per core (layer 0: shipped as two
# pre-permuted half-tables; layers 1-2: two AllGathers per boundary), so the
# per-edge source gathers are local HBM reads (int16 indices per half).
#
# Per core, per layer, for each chunk of 128 destination nodes:
#   - dma_gather the (dst-sorted) edges' source rows from the bf16 table
#   - build a selection matrix S[e, j] = (dst_local[e] == j) * 1/deg per
#     128-edge tile with one DVE tensor_scalar (bf16, 2x mode) against an
#     iota row, and accumulate aggrT += msg_tile^T-contract S on the PE into
#     PSUM ([feat, node] transposed mean aggregation)
#   - transpose the chunk's own rows with the PE, then
#     h = relu(aggrT^T @ w_l + own @ w_r), ScalarE draining PSUM into the
#     next layer's SBUF-resident own-feature table
# Collective overlap: the table halves are chunk-aligned; the small A
# half's h rows finish first and AllGather mid-layer (hidden under the
# remaining chunks), while the big B half's AllGather fires at layer end
# and hides under the next layer's pass-1, which processes only A-sourced
# tiles (partial aggregations parked in SBUF and re-injected into PSUM via
# an identity matmul in pass 2). Own-row transposes and the w_r matmul also
# run in pass 1 (they need no table). Layer 0 runs single-pass (its tables
# are inputs). The S selection matrices depend only on the edge structure,
# so they are built once in layer 0 (DVE), spilled to DRAM, and DMA-
# reloaded in layers 1-2 instead of rebuilt. Host-side preprocessing (edge
# sort/partition, padding to an SPMD-uniform tile layout, index tables) is
# numpy; inputs shipped minimal (bf16 aux, idx tables wrapped [16, cols]
# and replicated on device, bf16 output upcast on host).
import os
import sys

import numpy as np

for _p in ("/opt/trn_rl_repo", "/root/.axon_site/_ro/trn_rl_repo"):
    if _p not in sys.path and os.path.isdir(_p):
        sys.path.append(_p)

import ml_dtypes  # noqa: E402

from concourse import bacc, bass, mybir, tile  # noqa: E402
from concourse.bass_utils import axon_active, run_bass_kernel_spmd  # noqa: E402
from concourse.masks import make_identity  # noqa: E402

P = 128
BF16 = mybir.dt.bfloat16
F32 = mybir.dt.float32
I16 = mybir.dt.int16
NP_BF16 = ml_dtypes.bfloat16


class Cfg:
    def __init__(self, n_nodes=50000, n_cores=8, d_in=128, d_hid=128, d_out=64,
                 split_chunks=20):
        assert n_nodes % n_cores == 0
        self.N = n_nodes
        self.NC = n_cores
        self.NPC = n_nodes // n_cores
        self.D_IN = d_in
        self.D_HID = d_hid
        self.D_OUT = d_out
        self.NCH = (self.NPC + P - 1) // P
        self.SC = split_chunks              # chunks in half A
        self.RA = split_chunks * P          # local rows in half A
        self.RB = self.NPC - self.RA        # local rows in half B
        self.TA = self.RA * n_cores         # table-A rows
        self.TB = self.RB * n_cores         # table-B rows
        assert self.TA < 2**15 and self.TB < 2**15


def preprocess(cfg: Cfg, src: np.ndarray, dst: np.ndarray):
    """Sort edges by dst, group per (chunk, table-half), pad to SPMD-uniform
    tiles, emit SBUF-layout index/dst/invd arrays ([16, cols] idx wrap)."""
    N, NPC, NCH, NC = cfg.N, cfg.NPC, cfg.NCH, cfg.NC
    RA, RB = cfg.RA, cfg.RB

    deg = np.bincount(dst, minlength=N)
    invd_node = (1.0 / np.maximum(deg, 1)).astype(np.float32)

    order = np.argsort(dst, kind="stable")
    s_src = src[order]
    s_dst = dst[order]

    lo_cnt = np.zeros((NC, NCH), dtype=np.int64)
    hi_cnt = np.zeros((NC, NCH), dtype=np.int64)
    bounds = np.empty((NC, NCH + 1), dtype=np.int64)
    for i in range(NC):
        for c in range(NCH + 1):
            bounds[i, c] = np.searchsorted(s_dst, i * NPC + min(c * P, NPC), "left")
        for c in range(NCH):
            e0, e1 = bounds[i, c], bounds[i, c + 1]
            in_a = (s_src[e0:e1] % NPC) < RA
            lo_cnt[i, c] = int(np.count_nonzero(in_a))
            hi_cnt[i, c] = (e1 - e0) - lo_cnt[i, c]

    cdiv = lambda a, b: -(-a // b)
    T_a = [int(cdiv(int(lo_cnt[:, c].max()), P)) for c in range(NCH)]
    T_b = [int(cdiv(int(hi_cnt[:, c].max()), P)) for c in range(NCH)]
    TT = sum(T_a) + sum(T_b)

    # slot layout: [all chunks' A-tiles in chunk order] then [all B-tiles]
    # (pass 1 = A tiles, pass 2 = B tiles)
    a_off = np.cumsum([0] + T_a)  # tile offsets within the A region
    b_off = np.cumsum([0] + T_b)
    NTA, NTB = a_off[-1], b_off[-1]
    SLOTS = TT * P

    idx16 = np.zeros((NC, SLOTS), dtype=np.int16)
    dstloc = np.full((NC, SLOTS), -1.0, dtype=np.float32)
    invd = np.zeros((NC, SLOTS), dtype=np.float32)

    for i in range(NC):
        for c in range(NCH):
            e0, e1 = bounds[i, c], bounds[i, c + 1]
            seg_src = s_src[e0:e1]
            seg_dst = s_dst[e0:e1]
            s_i, s_r = seg_src // NPC, seg_src % NPC
            in_a = s_r < RA
            tidx = np.where(in_a, s_i * RA + s_r, s_i * RB + (s_r - RA))
            base = i * NPC + c * P
            for sel, off_tiles in ((True, a_off[c]), (False, NTA + b_off[c])):
                m = in_a == sel
                n = int(m.sum())
                pos = off_tiles * P
                idx16[i, pos : pos + n] = tidx[m].astype(np.int16)
                dstloc[i, pos : pos + n] = (seg_dst[m] - base).astype(np.float32)
                invd[i, pos : pos + n] = invd_node[seg_dst[m]]

    # idx wrap: slot j -> [j % 16, j // 16]; shipped once (16 rows), the
    # kernel replicates it to all eight 16-partition groups.
    idx16_sb = np.ascontiguousarray(
        idx16.reshape(NC, SLOTS // 16, 16).transpose(0, 2, 1)
    )  # [NC, 16, SLOTS//16]
    dst_sb = np.ascontiguousarray(
        dstloc.reshape(NC, TT, P).transpose(0, 2, 1)
    ).astype(NP_BF16)
    inv_sb = np.ascontiguousarray(
        invd.reshape(NC, TT, P).transpose(0, 2, 1)
    ).astype(NP_BF16)

    return T_a, T_b, idx16_sb, dst_sb, inv_sb


def build(cfg: Cfg, T_a, T_b, n_gather_queues=1, maxi=896, repeat=1,
          skip_collectives=False, s_pool_mod=0, s_rebuild=False):
    N, NPC, NCH, NC = cfg.N, cfg.NPC, cfg.NCH, cfg.NC
    D_IN, D_HID, D_OUT = cfg.D_IN, cfg.D_HID, cfg.D_OUT
    SC, RA, RB, TA, TB = cfg.SC, cfg.RA, cfg.RB, cfg.TA, cfg.TB
    TT = sum(T_a) + sum(T_b)
    NTA = sum(T_a)
    TAMAX = max(T_a)
    TBMAX = max(T_b)
    a_off = np.cumsum([0] + T_a)
    b_off = np.cumsum([0] + T_b)

    nc = bacc.Bacc(
        "TRN2",
        target_bir_lowering=False,
        debug=not axon_active(),
        num_devices=NC,
        num_swdge_queues=n_gather_queues,
    )

    xown_d = nc.dram_tensor("xown", [NPC, D_IN], BF16, kind="ExternalInput")
    xa_d = nc.dram_tensor("xa", [TA, D_IN], BF16, kind="ExternalInput")
    xb_d = nc.dram_tensor("xb", [TB, D_IN], BF16, kind="ExternalInput")
    idx_d = nc.dram_tensor("idx16", [16, TT * 8], I16, kind="ExternalInput")
    dst_d = nc.dram_tensor("dstloc", [P, TT], BF16, kind="ExternalInput")
    inv_d = nc.dram_tensor("invd", [P, TT], BF16, kind="ExternalInput")
    w_d = {}
    for li, (din, dout) in enumerate(((D_IN, D_HID), (D_HID, D_HID), (D_HID, D_OUT))):
        w_d[f"wl{li}"] = nc.dram_tensor(f"wl{li}", [din, dout], BF16, kind="ExternalInput")
        w_d[f"wr{li}"] = nc.dram_tensor(f"wr{li}", [din, dout], BF16, kind="ExternalInput")
    out_d = nc.dram_tensor("out", [NPC, D_OUT], BF16, kind="ExternalOutput")

    from contextlib import ExitStack

    with tile.TileContext(nc) as tc, ExitStack() as stk:
        const = stk.enter_context(tc.tile_pool(name="const", bufs=1))
        iota_b = const.tile([P, P], BF16, name="iota_b")
        iota_i = const.tile([P, P], mybir.dt.int32, name="iota_i")
        nc.gpsimd.iota(iota_i[:], pattern=[[1, P]], base=0, channel_multiplier=0)
        nc.vector.tensor_copy(iota_b[:], iota_i[:])
        ident = const.tile([P, P], BF16, name="ident")
        make_identity(nc, ident[:])

        idx_t = const.tile([P, TT * 8], I16, name="idx_t")
        for g in range(8):
            nc.sync.dma_start(idx_t[16 * g : 16 * (g + 1), :], idx_d[:])
        dst_b = const.tile([P, TT], BF16, name="dst_b")
        nc.sync.dma_start(dst_b[:], dst_d[:])
        inv_b = const.tile([P, TT], BF16, name="inv_b")
        nc.sync.dma_start(inv_b[:], inv_d[:])
        dst_t = const.tile([P, TT], F32, name="dst_t")
        nc.vector.tensor_copy(dst_t[:], dst_b[:])
        inv_t = const.tile([P, TT], F32, name="inv_t")
        nc.vector.tensor_copy(inv_t[:], inv_b[:])

        w_t = {}
        for k, d in w_d.items():
            w_t[k] = const.tile(list(d.shape), d.dtype, name=f"{k}_t")
            nc.sync.dma_start(w_t[k][:], d[:])

        # own-feature tables (SBUF-resident), one per layer boundary
        own_all = [
            const.tile([P, NCH, D_IN if li == 0 else D_HID], BF16,
                       name=f"own_all{li}")
            for li in range(3)
        ]
        # layer-0 own rows: row r -> [r % 128, r // 128, :]
        nc.sync.dma_start(
            own_all[0][:, : NPC // P, :],
            xown_d[0 : (NPC // P) * P, :].rearrange("(c p) f -> p c f", p=P),
        )
        if NPC % P:
            nc.sync.dma_start(
                own_all[0][: NPC % P, NPC // P, :],
                xown_d[(NPC // P) * P :, :],
            )
        out_all = const.tile([P, NCH, D_OUT], BF16, name="out_all")
        partials = [
            const.tile([P, NCH, P], BF16, name=f"partial{i}") for i in range(2)
        ]
        hparts = [
            const.tile([P, NCH, D_HID], BF16, name=f"hpart{i}") for i in range(2)
        ]

        dram = stk.enter_context(tc.tile_pool(name="dram", bufs=1, space="DRAM"))

        msgp = stk.enter_context(tc.tile_pool(name="msg", bufs=6))
        sgp = stk.enter_context(tc.tile_pool(name="sel", bufs=4))
        wk = stk.enter_context(tc.tile_pool(name="wk", bufs=4))
        ps_ag = stk.enter_context(tc.tile_pool(name="ps_ag", bufs=3, space="PSUM"))
        ps_tr = stk.enter_context(tc.tile_pool(name="ps_tr", bufs=2, space="PSUM"))
        ps_h = stk.enter_context(tc.tile_pool(name="ps_h", bufs=2, space="PSUM"))

        gq = [0]
        nreg = {}

        def gather(out_ap, tab_ap, col0, n_idx):
            qn = gq[0] % n_gather_queues
            gq[0] += 1
            for off in range(0, n_idx, maxi):
                n = min(maxi, n_idx - off)
                t0, t1 = off // P, (off + n) // P
                if n not in nreg:
                    nreg[n] = nc.gpsimd.to_reg(n)
                nc.gpsimd.dma_gather(
                    out_ap[:, t0:t1, :],
                    tab_ap,
                    idx_t[:, col0 + off // 16 : col0 + (off + n) // 16],
                    num_idxs=n,
                    num_idxs_reg=nreg[n],
                    elem_size=out_ap.shape[-1],
                    queue_num=qn,
                )

        scount = [0]

        def sbuild_into(s_ap, til):
            scount[0] += 1
            eng = (
                nc.gpsimd
                if s_pool_mod and scount[0] % s_pool_mod == 0
                else nc.vector
            )
            eng.tensor_scalar(
                s_ap,
                iota_b[:],
                dst_t[:, til : til + 1],
                inv_t[:, til : til + 1],
                mybir.AluOpType.is_equal,
                mybir.AluOpType.mult,
            )

        # S selection matrices depend only on (dstloc, invd), which are
        # layer-independent: build them once (first layer of the first rep),
        # spill to DRAM, and DMA-reload afterwards instead of re-running the
        # DVE builds.
        s_store = dram.tile([P, TT * P], BF16, name="s_store")
        SGMAX = max(TAMAX, TBMAX)

        def s_group(build, til0, T):
            sg = sgp.tile([P, SGMAX, P], BF16, tag="Sg")
            if s_rebuild:
                for t in range(T):
                    sbuild_into(sg[:, t, :], til0 + t)
                return sg
            view = s_store[:, til0 * P : (til0 + T) * P].rearrange(
                "p (t j) -> p t j", j=P
            )
            if build:
                for t in range(T):
                    sbuild_into(sg[:, t, :], til0 + t)
                nc.sync.dma_start(view, sg[:, :T, :])
            else:
                nc.sync.dma_start(sg[:, :T, :], view)
            return sg

        for rep in range(repeat):
            # per-rep DRAM tables (AllGather outputs + inputs)
            h_own = [
                dram.tile([NPC, D_HID], BF16, name=f"h_own{li}_r{rep}")
                for li in range(2)
            ]
            tabs = [(xa_d, xb_d)]  # per layer: (tab_a, tab_b)
            for li in range(1, 3):
                tabs.append(
                    (
                        dram.tile([TA, D_HID], BF16, name=f"tabA{li}_r{rep}",
                                  addr_space="Shared"),
                        dram.tile([TB, D_HID], BF16, name=f"tabB{li}_r{rep}",
                                  addr_space="Shared"),
                    )
                )
            out_t = out_d if rep == repeat - 1 else dram.tile(
                [NPC, D_OUT], BF16, name=f"oscr_r{rep}"
            )

            def ag(ins_ap, out_tile):
                if skip_collectives:
                    return
                nc.gpsimd.collective_compute(
                    "AllGather",
                    mybir.AluOpType.bypass,
                    replica_groups=[list(range(NC))],
                    ins=[ins_ap],
                    outs=[out_tile.opt()],
                )

            # per-tile-group accessors: half 0 = A, half 1 = B
            def tiles_of(half, c):
                return T_a[c] if half == 0 else T_b[c]

            def til0_of(half, c):
                return a_off[c] if half == 0 else NTA + b_off[c]

            def msgshape(half):
                return TAMAX if half == 0 else TBMAX

            def emit_htab(layer, half, h_own, nxt, dout):
                """Batched h write for one table half + its AllGather."""
                if half == 0:
                    nc.sync.dma_start(
                        h_own[0:RA, :].rearrange("(c p) f -> p c f", p=P),
                        nxt[:, 0:SC, :dout],
                    )
                    ag(h_own[0:RA, :], tabs[layer + 1][0])
                else:
                    nc.sync.dma_start(
                        h_own[RA : RA + (NCH - SC - 1) * P, :]
                        .rearrange("(c p) f -> p c f", p=P),
                        nxt[:, SC : NCH - 1, :dout],
                    )
                    nc.sync.dma_start(
                        h_own[(NCH - 1) * P : NPC, :],
                        nxt[: NPC - (NCH - 1) * P, NCH - 1, :dout],
                    )
                    ag(h_own[RA:NPC, :], tabs[layer + 1][1])

            # The small A half always AllGathers early (fires after its SC
            # chunks, delaying the serial collective chain least); the big B
            # half AllGathers late and hides under the next layer's pass 1,
            # which only touches A-sourced tiles.
            early = [0, 0]          # early half per boundary layer (0, 1)
            p1_half = [0, 0, 0]     # pass-1 tile half per layer
            for layer in range(3):
                din = D_IN if layer == 0 else D_HID
                dout = D_HID if layer < 2 else D_OUT
                wl_t = w_t[f"wl{layer}"]
                wr_t = w_t[f"wr{layer}"]
                own = own_all[layer]
                h1 = p1_half[layer]
                h2 = 1 - h1
                tab1 = tabs[layer][h1]
                tab2 = tabs[layer][h2]
                if layer < 2:
                    # chunk order: the early half's chunks first
                    p2_order = (
                        list(range(NCH)) if early[layer] == 0
                        else list(range(SC, NCH)) + list(range(SC))
                    )
                else:
                    p2_order = list(range(NCH))
                partial = partials[layer % 2]
                hpart = hparts[layer % 2]
                merged = layer == 0  # both tables ready at start: single pass

                # ---- pass 1: h1 tiles -> partial aggT; own -> hpart ----
                if not merged:
                    for c in range(NCH):
                        T1 = tiles_of(h1, c)
                        if T1:
                            msg_t = msgp.tile([P, msgshape(h1), din], BF16,
                                              tag="msg1")
                            gather(msg_t[:, :T1, :], tab1[:],
                                   til0_of(h1, c) * 8, T1 * P)
                            sg = s_group(False, til0_of(h1, c), T1)
                            agg_ps = ps_ag.tile([P, P], F32, tag="agg")
                            for t in range(T1):
                                nc.tensor.matmul(
                                    agg_ps[:],
                                    lhsT=msg_t[:, t, :],
                                    rhs=sg[:, t, :],
                                    start=(t == 0),
                                    stop=(t == T1 - 1),
                                )
                            nc.scalar.activation(
                                partial[:, c, :], agg_ps[:],
                                mybir.ActivationFunctionType.Copy,
                            )
                        # own-row transpose + lin_r matmul (table-free work)
                        xT_ps = ps_tr.tile([P, P], BF16, tag="xT")
                        nc.tensor.transpose(xT_ps[:], own[:, c, :din], ident[:])
                        xT = wk.tile([P, P], BF16, tag="xT_sb")
                        nc.scalar.activation(
                            xT[:], xT_ps[:], mybir.ActivationFunctionType.Copy
                        )
                        hp_ps = ps_h.tile([P, dout], F32, tag="h")
                        nc.tensor.matmul(hp_ps[:], lhsT=xT[:, :din], rhs=wr_t[:],
                                         start=True, stop=True)
                        nc.scalar.activation(
                            hpart[:, c, :dout], hp_ps[:],
                            mybir.ActivationFunctionType.Copy,
                        )

                # ---- pass 2: h2 tiles + weights + output ----
                done = [0, 0]  # chunks completed per half
                for c in p2_order:
                    T2 = tiles_of(h2, c)
                    agg_ps = ps_ag.tile([P, P], F32, tag="agg")
                    started = False
                    if merged:
                        T1 = tiles_of(h1, c)
                        if T1:
                            msg_t = msgp.tile([P, msgshape(h1), din], BF16,
                                              tag="msg1")
                            gather(msg_t[:, :T1, :], tab1[:],
                                   til0_of(h1, c) * 8, T1 * P)
                            sg = s_group(rep == 0, til0_of(h1, c), T1)
                            for t in range(T1):
                                nc.tensor.matmul(
                                    agg_ps[:],
                                    lhsT=msg_t[:, t, :],
                                    rhs=sg[:, t, :],
                                    start=(t == 0),
                                    stop=False,
                                )
                            started = True
                    elif tiles_of(h1, c) > 0:
                        nc.tensor.matmul(
                            agg_ps[:], lhsT=ident[:], rhs=partial[:, c, :],
                            start=True, stop=(T2 == 0),
                        )
                        started = True
                    if T2:
                        msg_t = msgp.tile([P, msgshape(h2), din], BF16, tag="msg2")
                        gather(msg_t[:, :T2, :], tab2[:], til0_of(h2, c) * 8,
                               T2 * P)
                        sg = s_group(merged and rep == 0, til0_of(h2, c), T2)
                        for t in range(T2):
                            nc.tensor.matmul(
                                agg_ps[:],
                                lhsT=msg_t[:, t, :],
                                rhs=sg[:, t, :],
                                start=not started and t == 0,
                                stop=(t == T2 - 1),
                            )
                    aggT = wk.tile([P, P], BF16, tag="aggT")
                    nc.scalar.activation(
                        aggT[:], agg_ps[:], mybir.ActivationFunctionType.Copy
                    )

                    h_ps = ps_h.tile([P, dout], F32, tag="h")
                    if merged:
                        xT_ps = ps_tr.tile([P, P], BF16, tag="xT")
                        nc.tensor.transpose(xT_ps[:], own[:, c, :din], ident[:])
                        xT = wk.tile([P, P], BF16, tag="xT_sb")
                        nc.scalar.activation(
                            xT[:], xT_ps[:], mybir.ActivationFunctionType.Copy
                        )
                        nc.tensor.matmul(h_ps[:], lhsT=aggT[:], rhs=wl_t[:],
                                         start=True, stop=False)
                        nc.tensor.matmul(h_ps[:], lhsT=xT[:, :din], rhs=wr_t[:],
                                         start=False, stop=True)
                    else:
                        nc.tensor.matmul(h_ps[:], lhsT=ident[:],
                                         rhs=hpart[:, c, :dout],
                                         start=True, stop=False)
                        nc.tensor.matmul(h_ps[:], lhsT=aggT[:], rhs=wl_t[:],
                                         start=False, stop=True)

                    if layer < 2:
                        nc.scalar.activation(
                            own_all[layer + 1][:, c, :dout], h_ps[:],
                            mybir.ActivationFunctionType.Relu,
                        )
                    else:
                        nc.scalar.activation(
                            out_all[:, c, :], h_ps[:],
                            mybir.ActivationFunctionType.Copy,
                        )

                    if layer < 2:
                        hf = 0 if c < SC else 1
                        done[hf] += 1
                        if done[hf] == (SC if hf == 0 else NCH - SC):
                            emit_htab(layer, hf, h_own[layer],
                                      own_all[layer + 1], dout)

                # final output write
                if layer == 2:
                    nc.sync.dma_start(
                        out_t[0 : (NCH - 1) * P, :].rearrange(
                            "(c p) f -> p c f", p=P),
                        out_all[:, 0 : NCH - 1, :],
                    )
                    nc.sync.dma_start(
                        out_t[(NCH - 1) * P : NPC, :],
                        out_all[: NPC - (NCH - 1) * P, NCH - 1, :],
                    )

    nc.compile()
    return nc


def run(cfg: Cfg, inputs: dict, trace=False, tmpdir=None, **bkw):
    x = np.asarray(inputs["x"], dtype=np.float32)
    ei = np.asarray(inputs["edge_index"])
    src = ei[0].astype(np.int64)
    dst = ei[1].astype(np.int64)

    T_a, T_b, idx16_sb, dst_sb, inv_sb = preprocess(cfg, src, dst)
    nc = build(cfg, T_a, T_b, **bkw)

    x_bf = x.astype(NP_BF16)
    x3 = x_bf.reshape(cfg.NC, cfg.NPC, cfg.D_IN)
    xa = np.ascontiguousarray(x3[:, : cfg.RA, :]).reshape(cfg.TA, cfg.D_IN)
    xb = np.ascontiguousarray(x3[:, cfg.RA :, :]).reshape(cfg.TB, cfg.D_IN)
    in_maps = []
    for i in range(cfg.NC):
        m = {
            "xown": np.ascontiguousarray(x_bf[i * cfg.NPC : (i + 1) * cfg.NPC]),
            "xa": xa,
            "xb": xb,
            "idx16": idx16_sb[i],
            "dstloc": dst_sb[i],
            "invd": inv_sb[i],
        }
        for li in range(3):
            m[f"wl{li}"] = np.asarray(inputs[f"w_l{li}"], np.float32).astype(NP_BF16)
            m[f"wr{li}"] = np.asarray(inputs[f"w_r{li}"], np.float32).astype(NP_BF16)
        in_maps.append(m)

    results = run_bass_kernel_spmd(
        nc, in_maps, core_ids=list(range(cfg.NC)), trace=trace, tmpdir=tmpdir
    )
    outs = [np.asarray(r["out"]).astype(np.float32) for r in results.results]
    return np.concatenate(outs, axis=0), results, in_maps, nc


def kernel(**inputs) -> np.ndarray:
    cfg = Cfg()
    out, *_ = run(cfg, inputs)
    return out


# revision 29
# speedup vs baseline: 1.1010x; 1.0070x over previous
# GraphSAGE (3-layer, mean aggregation) on 8 Trainium2 NeuronCores.
#
# Sharding: nodes are split into 8 contiguous ranges (6250 per core); edges
# are partitioned by destination node so each core's scatter-adds stay local.
# Each layer's feature table is replicated per core (layer 0: shipped as two
# pre-permuted half-tables; layers 1-2: two AllGathers per boundary), so the
# per-edge source gathers are local HBM reads (int16 indices per half).
#
# Per core, per layer, for each chunk of 128 destination nodes:
#   - dma_gather the (dst-sorted) edges' source rows from the bf16 table
#   - build a selection matrix S[e, j] = (dst_local[e] == j) * 1/deg per
#     128-edge tile with one DVE tensor_scalar (bf16, 2x mode) against an
#     iota row, and accumulate aggrT += msg_tile^T-contract S on the PE into
#     PSUM ([feat, node] transposed mean aggregation)
#   - transpose the chunk's own rows with the PE, then
#     h = relu(aggrT^T @ w_l + own @ w_r), ScalarE draining PSUM into the
#     next layer's SBUF-resident own-feature table
# Collective overlap: the table halves are chunk-aligned; the small A
# half's h rows finish first and AllGather mid-layer (hidden under the
# remaining chunks), while the big B half's AllGather fires at layer end
# and hides under the next layer's pass-1, which processes only A-sourced
# tiles (partial aggregations parked in SBUF and re-injected into PSUM via
# an identity matmul in pass 2). Own-row transposes and the w_r matmul also
# run in pass 1 (they need no table). Layer 0 runs single-pass (its tables
# are inputs). The S selection matrices depend only on the edge structure,
# so they are built once in layer 0 (DVE), spilled to DRAM, and DMA-
# reloaded in layers 1-2 instead of rebuilt. Host-side preprocessing (edge
# sort/partition, padding to an SPMD-uniform tile layout, index tables) is
# numpy; inputs shipped minimal (bf16 aux, idx tables wrapped [16, cols]
# and replicated on device, bf16 output upcast on host).
import os
import sys

import numpy as np

for _p in ("/opt/trn_rl_repo", "/root/.axon_site/_ro/trn_rl_repo"):
    if _p not in sys.path and os.path.isdir(_p):
        sys.path.append(_p)

import ml_dtypes  # noqa: E402

from concourse import bacc, bass, mybir, tile  # noqa: E402
from concourse.bass_utils import axon_active, run_bass_kernel_spmd  # noqa: E402
from concourse.masks import make_identity  # noqa: E402

P = 128
BF16 = mybir.dt.bfloat16
F32 = mybir.dt.float32
I16 = mybir.dt.int16
NP_BF16 = ml_dtypes.bfloat16


class Cfg:
    def __init__(self, n_nodes=50000, n_cores=8, d_in=128, d_hid=128, d_out=64,
                 split_chunks=20):
        assert n_nodes % n_cores == 0
        self.N = n_nodes
        self.NC = n_cores
        self.NPC = n_nodes // n_cores
        self.D_IN = d_in
        self.D_HID = d_hid
        self.D_OUT = d_out
        self.NCH = (self.NPC + P - 1) // P
        self.SC = split_chunks              # chunks in half A
        self.RA = split_chunks * P          # local rows in half A
        self.RB = self.NPC - self.RA        # local rows in half B
        self.TA = self.RA * n_cores         # table-A rows
        self.TB = self.RB * n_cores         # table-B rows
        assert self.TA < 2**15 and self.TB < 2**15


def preprocess(cfg: Cfg, src: np.ndarray, dst: np.ndarray):
    """Sort edges by dst, group per (chunk, table-half), pad to SPMD-uniform
    tiles, emit SBUF-layout index/dst/invd arrays ([16, cols] idx wrap)."""
    N, NPC, NCH, NC = cfg.N, cfg.NPC, cfg.NCH, cfg.NC
    RA, RB = cfg.RA, cfg.RB

    deg = np.bincount(dst, minlength=N)
    invd_node = (1.0 / np.maximum(deg, 1)).astype(np.float32)

    order = np.argsort(dst, kind="stable")
    s_src = src[order]
    s_dst = dst[order]

    lo_cnt = np.zeros((NC, NCH), dtype=np.int64)
    hi_cnt = np.zeros((NC, NCH), dtype=np.int64)
    bounds = np.empty((NC, NCH + 1), dtype=np.int64)
    for i in range(NC):
        for c in range(NCH + 1):
            bounds[i, c] = np.searchsorted(s_dst, i * NPC + min(c * P, NPC), "left")
        for c in range(NCH):
            e0, e1 = bounds[i, c], bounds[i, c + 1]
            in_a = (s_src[e0:e1] % NPC) < RA
            lo_cnt[i, c] = int(np.count_nonzero(in_a))
            hi_cnt[i, c] = (e1 - e0) - lo_cnt[i, c]

    cdiv = lambda a, b: -(-a // b)
    T_a = [int(cdiv(int(lo_cnt[:, c].max()), P)) for c in range(NCH)]
    T_b = [int(cdiv(int(hi_cnt[:, c].max()), P)) for c in range(NCH)]
    TT = sum(T_a) + sum(T_b)

    # slot layout: [all chunks' A-tiles in chunk order] then [all B-tiles]
    # (pass 1 = A tiles, pass 2 = B tiles)
    a_off = np.cumsum([0] + T_a)  # tile offsets within the A region
    b_off = np.cumsum([0] + T_b)
    NTA, NTB = a_off[-1], b_off[-1]
    SLOTS = TT * P

    idx16 = np.zeros((NC, SLOTS), dtype=np.int16)
    dstloc = np.full((NC, SLOTS), -1.0, dtype=np.float32)
    invd = np.zeros((NC, SLOTS), dtype=np.float32)

    for i in range(NC):
        for c in range(NCH):
            e0, e1 = bounds[i, c], bounds[i, c + 1]
            seg_src = s_src[e0:e1]
            seg_dst = s_dst[e0:e1]
            s_i, s_r = seg_src // NPC, seg_src % NPC
            in_a = s_r < RA
            tidx = np.where(in_a, s_i * RA + s_r, s_i * RB + (s_r - RA))
            base = i * NPC + c * P
            for sel, off_tiles in ((True, a_off[c]), (False, NTA + b_off[c])):
                m = in_a == sel
                n = int(m.sum())
                pos = off_tiles * P
                idx16[i, pos : pos + n] = tidx[m].astype(np.int16)
                dstloc[i, pos : pos + n] = (seg_dst[m] - base).astype(np.float32)
                invd[i, pos : pos + n] = invd_node[seg_dst[m]]

    # idx wrap: slot j -> [j % 16, j // 16]; shipped once (16 rows), the
    # kernel replicates it to all eight 16-partition groups.
    idx16_sb = np.ascontiguousarray(
        idx16.reshape(NC, SLOTS // 16, 16).transpose(0, 2, 1)
    )  # [NC, 16, SLOTS//16]
    dst_sb = np.ascontiguousarray(
        dstloc.reshape(NC, TT, P).transpose(0, 2, 1)
    ).astype(NP_BF16)
    inv_sb = np.ascontiguousarray(
        invd.reshape(NC, TT, P).transpose(0, 2, 1)
    ).astype(NP_BF16)

    return T_a, T_b, idx16_sb, dst_sb, inv_sb


def build(cfg: Cfg, T_a, T_b, n_gather_queues=4, maxi=896, repeat=1,
          skip_collectives=False, s_pool_mod=0, s_rebuild=False):
    N, NPC, NCH, NC = cfg.N, cfg.NPC, cfg.NCH, cfg.NC
    D_IN, D_HID, D_OUT = cfg.D_IN, cfg.D_HID, cfg.D_OUT
    SC, RA, RB, TA, TB = cfg.SC, cfg.RA, cfg.RB, cfg.TA, cfg.TB
    TT = sum(T_a) + sum(T_b)
    NTA = sum(T_a)
    TAMAX = max(T_a)
    TBMAX = max(T_b)
    a_off = np.cumsum([0] + T_a)
    b_off = np.cumsum([0] + T_b)

    nc = bacc.Bacc(
        "TRN2",
        target_bir_lowering=False,
        debug=not axon_active(),
        num_devices=NC,
        num_swdge_queues=n_gather_queues,
    )

    xown_d = nc.dram_tensor("xown", [NPC, D_IN], BF16, kind="ExternalInput")
    xa_d = nc.dram_tensor("xa", [TA, D_IN], BF16, kind="ExternalInput")
    xb_d = nc.dram_tensor("xb", [TB, D_IN], BF16, kind="ExternalInput")
    idx_d = nc.dram_tensor("idx16", [16, TT * 8], I16, kind="ExternalInput")
    dst_d = nc.dram_tensor("dstloc", [P, TT], BF16, kind="ExternalInput")
    inv_d = nc.dram_tensor("invd", [P, TT], BF16, kind="ExternalInput")
    w_d = {}
    for li, (din, dout) in enumerate(((D_IN, D_HID), (D_HID, D_HID), (D_HID, D_OUT))):
        w_d[f"wl{li}"] = nc.dram_tensor(f"wl{li}", [din, dout], BF16, kind="ExternalInput")
        w_d[f"wr{li}"] = nc.dram_tensor(f"wr{li}", [din, dout], BF16, kind="ExternalInput")
    out_d = nc.dram_tensor("out", [NPC, D_OUT], BF16, kind="ExternalOutput")

    from contextlib import ExitStack

    with tile.TileContext(nc) as tc, ExitStack() as stk:
        const = stk.enter_context(tc.tile_pool(name="const", bufs=1))
        iota_b = const.tile([P, P], BF16, name="iota_b")
        iota_i = const.tile([P, P], mybir.dt.int32, name="iota_i")
        nc.gpsimd.iota(iota_i[:], pattern=[[1, P]], base=0, channel_multiplier=0)
        nc.vector.tensor_copy(iota_b[:], iota_i[:])
        ident = const.tile([P, P], BF16, name="ident")
        make_identity(nc, ident[:])

        idx_t = const.tile([P, TT * 8], I16, name="idx_t")
        for g in range(8):
            nc.sync.dma_start(idx_t[16 * g : 16 * (g + 1), :], idx_d[:])
        dst_b = const.tile([P, TT], BF16, name="dst_b")
        nc.sync.dma_start(dst_b[:], dst_d[:])
        inv_b = const.tile([P, TT], BF16, name="inv_b")
        nc.sync.dma_start(inv_b[:], inv_d[:])
        dst_t = const.tile([P, TT], F32, name="dst_t")
        nc.vector.tensor_copy(dst_t[:], dst_b[:])
        inv_t = const.tile([P, TT], F32, name="inv_t")
        nc.vector.tensor_copy(inv_t[:], inv_b[:])

        w_t = {}
        for k, d in w_d.items():
            w_t[k] = const.tile(list(d.shape), d.dtype, name=f"{k}_t")
            nc.sync.dma_start(w_t[k][:], d[:])

        # own-feature tables (SBUF-resident), one per layer boundary
        own_all = [
            const.tile([P, NCH, D_IN if li == 0 else D_HID], BF16,
                       name=f"own_all{li}")
            for li in range(3)
        ]
        # layer-0 own rows: row r -> [r % 128, r // 128, :]
        nc.sync.dma_start(
            own_all[0][:, : NPC // P, :],
            xown_d[0 : (NPC // P) * P, :].rearrange("(c p) f -> p c f", p=P),
        )
        if NPC % P:
            nc.sync.dma_start(
                own_all[0][: NPC % P, NPC // P, :],
                xown_d[(NPC // P) * P :, :],
            )
        out_all = const.tile([P, NCH, D_OUT], BF16, name="out_all")
        partials = [
            const.tile([P, NCH, P], BF16, name=f"partial{i}") for i in range(2)
        ]
        hparts = [
            const.tile([P, NCH, D_HID], BF16, name=f"hpart{i}") for i in range(2)
        ]

        dram = stk.enter_context(tc.tile_pool(name="dram", bufs=1, space="DRAM"))

        msgp = stk.enter_context(tc.tile_pool(name="msg", bufs=6))
        sgp = stk.enter_context(tc.tile_pool(name="sel", bufs=4))
        wk = stk.enter_context(tc.tile_pool(name="wk", bufs=4))
        ps_ag = stk.enter_context(tc.tile_pool(name="ps_ag", bufs=3, space="PSUM"))
        ps_tr = stk.enter_context(tc.tile_pool(name="ps_tr", bufs=2, space="PSUM"))
        ps_h = stk.enter_context(tc.tile_pool(name="ps_h", bufs=2, space="PSUM"))

        gq = [0]
        nreg = {}

        def gather(out_ap, tab_ap, col0, n_idx):
            for off in range(0, n_idx, maxi):
                n = min(maxi, n_idx - off)
                t0, t1 = off // P, (off + n) // P
                if n not in nreg:
                    nreg[n] = nc.gpsimd.to_reg(n)
                nc.gpsimd.dma_gather(
                    out_ap[:, t0:t1, :],
                    tab_ap,
                    idx_t[:, col0 + off // 16 : col0 + (off + n) // 16],
                    num_idxs=n,
                    num_idxs_reg=nreg[n],
                    elem_size=out_ap.shape[-1],
                    queue_num=gq[0] % n_gather_queues,
                )
                gq[0] += 1

        scount = [0]

        def sbuild_into(s_ap, til):
            scount[0] += 1
            eng = (
                nc.gpsimd
                if s_pool_mod and scount[0] % s_pool_mod == 0
                else nc.vector
            )
            eng.tensor_scalar(
                s_ap,
                iota_b[:],
                dst_t[:, til : til + 1],
                inv_t[:, til : til + 1],
                mybir.AluOpType.is_equal,
                mybir.AluOpType.mult,
            )

        # S selection matrices depend only on (dstloc, invd), which are
        # layer-independent: build them once (first layer of the first rep),
        # spill to DRAM, and DMA-reload afterwards instead of re-running the
        # DVE builds.
        s_store = dram.tile([P, TT * P], BF16, name="s_store")
        SGMAX = max(TAMAX, TBMAX)

        def s_group(build, til0, T):
            sg = sgp.tile([P, SGMAX, P], BF16, tag="Sg")
            if s_rebuild:
                for t in range(T):
                    sbuild_into(sg[:, t, :], til0 + t)
                return sg
            view = s_store[:, til0 * P : (til0 + T) * P].rearrange(
                "p (t j) -> p t j", j=P
            )
            if build:
                for t in range(T):
                    sbuild_into(sg[:, t, :], til0 + t)
                nc.sync.dma_start(view, sg[:, :T, :])
            else:
                nc.sync.dma_start(sg[:, :T, :], view)
            return sg

        for rep in range(repeat):
            # per-rep DRAM tables (AllGather outputs + inputs)
            h_own = [
                dram.tile([NPC, D_HID], BF16, name=f"h_own{li}_r{rep}")
                for li in range(2)
            ]
            tabs = [(xa_d, xb_d)]  # per layer: (tab_a, tab_b)
            for li in range(1, 3):
                tabs.append(
                    (
                        dram.tile([TA, D_HID], BF16, name=f"tabA{li}_r{rep}",
                                  addr_space="Shared"),
                        dram.tile([TB, D_HID], BF16, name=f"tabB{li}_r{rep}",
                                  addr_space="Shared"),
                    )
                )
            out_t = out_d if rep == repeat - 1 else dram.tile(
                [NPC, D_OUT], BF16, name=f"oscr_r{rep}"
            )

            def ag(ins_ap, out_tile):
                if skip_collectives:
                    return
                nc.gpsimd.collective_compute(
                    "AllGather",
                    mybir.AluOpType.bypass,
                    replica_groups=[list(range(NC))],
                    ins=[ins_ap],
                    outs=[out_tile.opt()],
                )

            # per-tile-group accessors: half 0 = A, half 1 = B
            def tiles_of(half, c):
                return T_a[c] if half == 0 else T_b[c]

            def til0_of(half, c):
                return a_off[c] if half == 0 else NTA + b_off[c]

            def msgshape(half):
                return TAMAX if half == 0 else TBMAX

            def emit_htab(layer, half, h_own, nxt, dout):
                """Batched h write for one table half + its AllGather."""
                if half == 0:
                    nc.sync.dma_start(
                        h_own[0:RA, :].rearrange("(c p) f -> p c f", p=P),
                        nxt[:, 0:SC, :dout],
                    )
                    ag(h_own[0:RA, :], tabs[layer + 1][0])
                else:
                    nc.sync.dma_start(
                        h_own[RA : RA + (NCH - SC - 1) * P, :]
                        .rearrange("(c p) f -> p c f", p=P),
                        nxt[:, SC : NCH - 1, :dout],
                    )
                    nc.sync.dma_start(
                        h_own[(NCH - 1) * P : NPC, :],
                        nxt[: NPC - (NCH - 1) * P, NCH - 1, :dout],
                    )
                    ag(h_own[RA:NPC, :], tabs[layer + 1][1])

            # The small A half always AllGathers early (fires after its SC
            # chunks, delaying the serial collective chain least); the big B
            # half AllGathers late and hides under the next layer's pass 1,
            # which only touches A-sourced tiles.
            early = [0, 0]          # early half per boundary layer (0, 1)
            p1_half = [0, 0, 0]     # pass-1 tile half per layer
            for layer in range(3):
                din = D_IN if layer == 0 else D_HID
                dout = D_HID if layer < 2 else D_OUT
                wl_t = w_t[f"wl{layer}"]
                wr_t = w_t[f"wr{layer}"]
                own = own_all[layer]
                h1 = p1_half[layer]
                h2 = 1 - h1
                tab1 = tabs[layer][h1]
                tab2 = tabs[layer][h2]
                if layer < 2:
                    # chunk order: the early half's chunks first
                    p2_order = (
                        list(range(NCH)) if early[layer] == 0
                        else list(range(SC, NCH)) + list(range(SC))
                    )
                else:
                    p2_order = list(range(NCH))
                partial = partials[layer % 2]
                hpart = hparts[layer % 2]
                merged = layer == 0  # both tables ready at start: single pass

                # ---- pass 1: h1 tiles -> partial aggT; own -> hpart ----
                if not merged:
                    for c in range(NCH):
                        T1 = tiles_of(h1, c)
                        if T1:
                            msg_t = msgp.tile([P, msgshape(h1), din], BF16,
                                              tag="msg1")
                            gather(msg_t[:, :T1, :], tab1[:],
                                   til0_of(h1, c) * 8, T1 * P)
                            sg = s_group(False, til0_of(h1, c), T1)
                            agg_ps = ps_ag.tile([P, P], F32, tag="agg")
                            for t in range(T1):
                                nc.tensor.matmul(
                                    agg_ps[:],
                                    lhsT=msg_t[:, t, :],
                                    rhs=sg[:, t, :],
                                    start=(t == 0),
                                    stop=(t == T1 - 1),
                                )
                            nc.scalar.activation(
                                partial[:, c, :], agg_ps[:],
                                mybir.ActivationFunctionType.Copy,
                            )
                        # own-row transpose + lin_r matmul (table-free work)
                        xT_ps = ps_tr.tile([P, P], BF16, tag="xT")
                        nc.tensor.transpose(xT_ps[:], own[:, c, :din], ident[:])
                        xT = wk.tile([P, P], BF16, tag="xT_sb")
                        nc.scalar.activation(
                            xT[:], xT_ps[:], mybir.ActivationFunctionType.Copy
                        )
                        hp_ps = ps_h.tile([P, dout], F32, tag="h")
                        nc.tensor.matmul(hp_ps[:], lhsT=xT[:, :din], rhs=wr_t[:],
                                         start=True, stop=True)
                        nc.scalar.activation(
                            hpart[:, c, :dout], hp_ps[:],
                            mybir.ActivationFunctionType.Copy,
                        )

                # ---- pass 2: h2 tiles + weights + output ----
                done = [0, 0]  # chunks completed per half
                for c in p2_order:
                    T2 = tiles_of(h2, c)
                    agg_ps = ps_ag.tile([P, P], F32, tag="agg")
                    started = False
                    if merged:
                        T1 = tiles_of(h1, c)
                        if T1:
                            msg_t = msgp.tile([P, msgshape(h1), din], BF16,
                                              tag="msg1")
                            gather(msg_t[:, :T1, :], tab1[:],
                                   til0_of(h1, c) * 8, T1 * P)
                            sg = s_group(rep == 0, til0_of(h1, c), T1)
                            for t in range(T1):
                                nc.tensor.matmul(
                                    agg_ps[:],
                                    lhsT=msg_t[:, t, :],
                                    rhs=sg[:, t, :],
                                    start=(t == 0),
                                    stop=False,
                                )
                            started = True
                    elif tiles_of(h1, c) > 0:
                        nc.tensor.matmul(
                            agg_ps[:], lhsT=ident[:], rhs=partial[:, c, :],
                            start=True, stop=(T2 == 0),
                        )
                        started = True
                    if T2:
                        msg_t = msgp.tile([P, msgshape(h2), din], BF16, tag="msg2")
                        gather(msg_t[:, :T2, :], tab2[:], til0_of(h2, c) * 8,
                               T2 * P)
                        sg = s_group(merged and rep == 0, til0_of(h2, c), T2)
                        for t in range(T2):
                            nc.tensor.matmul(
                                agg_ps[:],
                                lhsT=msg_t[:, t, :],
                                rhs=sg[:, t, :],
                                start=not started and t == 0,
                                stop=(t == T2 - 1),
                            )
                    aggT = wk.tile([P, P], BF16, tag="aggT")
                    nc.scalar.activation(
                        aggT[:], agg_ps[:], mybir.ActivationFunctionType.Copy
                    )

                    h_ps = ps_h.tile([P, dout], F32, tag="h")
                    if merged:
                        xT_ps = ps_tr.tile([P, P], BF16, tag="xT")
                        nc.tensor.transpose(xT_ps[:], own[:, c, :din], ident[:])
                        xT = wk.tile([P, P], BF16, tag="xT_sb")
                        nc.scalar.activation(
                            xT[:], xT_ps[:], mybir.ActivationFunctionType.Copy
                        )
                        nc.tensor.matmul(h_ps[:], lhsT=aggT[:], rhs=wl_t[:],
                                         start=True, stop=False)
                        nc.tensor.matmul(h_ps[:], lhsT=xT[:, :din], rhs=wr_t[:],
                                         start=False, stop=True)
                    else:
                        nc.tensor.matmul(h_ps[:], lhsT=ident[:],
                                         rhs=hpart[:, c, :dout],
                                         start=True, stop=False)
                        nc.tensor.matmul(h_ps[:], lhsT=aggT[:], rhs=wl_t[:],
                                         start=False, stop=True)

                    if layer < 2:
                        nc.scalar.activation(
                            own_all[layer + 1][:, c, :dout], h_ps[:],
                            mybir.ActivationFunctionType.Relu,
                        )
                    else:
                        nc.scalar.activation(
                            out_all[:, c, :], h_ps[:],
                            mybir.ActivationFunctionType.Copy,
                        )

                    if layer < 2:
                        hf = 0 if c < SC else 1
                        done[hf] += 1
                        if done[hf] == (SC if hf == 0 else NCH - SC):
                            emit_htab(layer, hf, h_own[layer],
                                      own_all[layer + 1], dout)

                # final output write
                if layer == 2:
                    nc.sync.dma_start(
                        out_t[0 : (NCH - 1) * P, :].rearrange(
                            "(c p) f -> p c f", p=P),
                        out_all[:, 0 : NCH - 1, :],
                    )
                    nc.sync.dma_start(
                        out_t[(NCH - 1) * P : NPC, :],
                        out_all[: NPC - (NCH - 1) * P, NCH - 1, :],
                    )

    nc.compile()
    return nc


def run(cfg: Cfg, inputs: dict, trace=False, tmpdir=None, **bkw):
    x = np.asarray(inputs["x"], dtype=np.float32)
    ei = np.asarray(inputs["edge_index"])
    src = ei[0].astype(np.int64)
    dst = ei[1].astype(np.int64)

    T_a, T_b, idx16_sb, dst_sb, inv_sb = preprocess(cfg, src, dst)
    nc = build(cfg, T_a, T_b, **bkw)

    x_bf = x.astype(NP_BF16)
    x3 = x_bf.reshape(cfg.NC, cfg.NPC, cfg.D_IN)
    xa = np.ascontiguousarray(x3[:, : cfg.RA, :]).reshape(cfg.TA, cfg.D_IN)
    xb = np.ascontiguousarray(x3[:, cfg.RA :, :]).reshape(cfg.TB, cfg.D_IN)
    in_maps = []
    for i in range(cfg.NC):
        m = {
            "xown": np.ascontiguousarray(x_bf[i * cfg.NPC : (i + 1) * cfg.NPC]),
            "xa": xa,
            "xb": xb,
            "idx16": idx16_sb[i],
            "dstloc": dst_sb[i],
            "invd": inv_sb[i],
        }
        for li in range(3):
            m[f"wl{li}"] = np.asarray(inputs[f"w_l{li}"], np.float32).astype(NP_BF16)
            m[f"wr{li}"] = np.asarray(inputs[f"w_r{li}"], np.float32).astype(NP_BF16)
        in_maps.append(m)

    results = run_bass_kernel_spmd(
        nc, in_maps, core_ids=list(range(cfg.NC)), trace=trace, tmpdir=tmpdir
    )
    outs = [np.asarray(r["out"]).astype(np.float32) for r in results.results]
    return np.concatenate(outs, axis=0), results, in_maps, nc


def kernel(**inputs) -> np.ndarray:
    cfg = Cfg()
    out, *_ = run(cfg, inputs)
    return out
